# revision 2
# baseline (speedup 1.0000x reference)
"""BitLinear inference kernel for Trainium2 (8 NeuronCores, column-parallel).

Math (per reference):
  s[t]   = max(|x[t,:]|) clipped to >= 1e-5          (per-token scale)
  xq     = round(x / s * 127)  (round-half-even)      (int values in [-127,127])
  out    = (xq @ w_ternary.T) * (s * weight_scale / 127)

The integer matmul xq @ w.T is EXACT in bf16 x bf16 -> fp32 PSUM:
xq in [-127,127] and w in {-1,0,1} are exactly representable in bf16,
products are exact, and partial sums are < 2^24 so fp32 accumulation is
exact. Per-token dequant scale is applied to the fp32 PSUM output.

Sharding: column-parallel. weight rows (out_features) are sharded 8 ways;
x is replicated; outputs are concatenated on host along out_features.
The weight shard is shipped host-transposed AND pre-cast to bf16
([in_f, of_shard]) so it can be DMA'd straight into the resident SBUF
weight tiles — no on-device dequant/cast on the startup critical path.
Weight DMAs ride the ACT HWDGE ring (nc.scalar) so they don't
head-of-line-block the SP ring that carries x loads, xbar transposes
and output stores.

Per-core pipeline, per 128-token tile:
  DMA   x tile in (2 halves), per-tile DVE quant (abs-max reduce,
        reciprocal, mult+magic-add, magic-sub -> bf16),
  DMA   xbar transpose SBUF->SBUF (bf16) into [128, 32, 128] lhsT chunks,
  PE    32 LDW+128 matmuls (N=512) accumulating [128 tok, 2048 of] fp32
        across 2 double-buffered PSUM tiles (8 banks),
  ACT   per-token-scale eviction (activation Copy, scale=[128,1] AP),
  DMA   store.

Correctness vs the fp32 jax reference: norm relative error 2.3e-05
(from inv=127*(1/s) vs the reference's x/s*127 double-rounding; the
integer matmul itself is exact).
"""

import numpy as np
import ml_dtypes

import concourse.bass as bass
import concourse.mybir as mybir
import concourse.tile as tile
from concourse import bacc

P = 128
MAGIC = 12582912.0  # 1.5 * 2**23: (v + MAGIC) - MAGIC == round-half-even(v) for |v|<=2^21

# problem shapes (hardcoded per contract)
B, S, IN_F, OUT_F = 4, 2048, 4096, 16384
N_CORES = 8
TOKENS = B * S
OF_SHARD = OUT_F // N_CORES


def build_program(tokens=TOKENS, in_f=IN_F, of=OF_SHARD, n_devices=N_CORES,
                  debug=False, ns=512, reps=1, timing=False,
                  timing_full=False, deep=False, variant="full"):
    """Build the SPMD single-core program. Returns the compiled Bacc object.

    timing=True makes the big tensors internal (nothing shipped over the
    wire) and adds a tiny external in/out pair; reps>1 wraps the token loop
    in a hardware For_i so per-iteration time can be measured as a slope.
    timing_full=True additionally moves the weight load inside the rep
    loop, so the slope approximates full per-invocation device time
    (weight DMA included) rather than steady-state-tiles-only.
    """
    TT = tokens // P      # token tiles
    KC = in_f // P        # contraction chunks
    NOF = of // ns        # psum column slices
    XH = in_f // 2        # x staged in halves to save SBUF

    nc = bacc.Bacc("TRN2", target_bir_lowering=False, debug=debug,
                   num_devices=n_devices)

    big_kind = "Internal" if timing else "ExternalInput"
    xf = nc.dram_tensor("x", [tokens, in_f], mybir.dt.float32,
                        kind=big_kind).ap()
    wt = nc.dram_tensor("wt", [in_f, of], mybir.dt.bfloat16,
                        kind=big_kind).ap()
    ws = nc.dram_tensor("ws", [P, 1], mybir.dt.float32,
                        kind="ExternalInput").ap()
    out = nc.dram_tensor(
        "out", [tokens, of], mybir.dt.float32,
        kind="Internal" if timing else "ExternalOutput").ap()
    tiny = None
    if timing:
        tiny = nc.dram_tensor("tiny", [P, 1], mybir.dt.float32,
                              kind="ExternalOutput").ap()

    xf3 = xf.rearrange("(tt p) f -> tt p f", p=P)
    wt3 = wt.rearrange("(kc p) o -> kc p o", p=P)
    out3 = out.rearrange("(tt p) o -> tt p o", p=P)

    with tile.TileContext(nc) as tc:
        with (
            tc.tile_pool(name="consts", bufs=1) as consts,
            tc.tile_pool(name="wpool", bufs=1) as wpool,
            tc.tile_pool(name="stage", bufs=3) as stage,
            tc.tile_pool(name="xqp", bufs=2 if deep else 1) as xqp,
            tc.tile_pool(name="xqtp", bufs=3 if deep else 2) as xqtp,
            tc.tile_pool(name="outp", bufs=2) as outp,
            tc.tile_pool(name="scal", bufs=3) as scal,
            tc.tile_pool(name="psum", bufs=2, space="PSUM") as psum,
        ):
            wsb = consts.tile([P, 1], mybir.dt.float32)
            nc.sync.dma_start(wsb[:], ws[:])

            # tile 0's x loads first so they land at the SP queue head
            pre_x = []
            if reps == 1 and variant != "mm":
                for h in range(2):
                    xt = stage.tile([P, XH], mybir.dt.float32, tag="stage",
                                    name=f"prex{h}")
                    nc.sync.dma_start(xt[:], xf3[0][:, h * XH:(h + 1) * XH])
                    pre_x.append(xt)

            # ---- weights: bf16 [in_f, of] DMA'd straight into resident
            # SBUF chunks on the ACT HWDGE ring (keeps SP ring free for
            # the x/transpose/store pipeline).
            wks = []

            def load_weights():
                del wks[:]
                for k in range(KC):
                    wk = wpool.tile([P, of], mybir.dt.bfloat16, tag=f"wk{k}",
                                    name=f"wk{k}")
                    nc.scalar.dma_start(wk[:], wt3[k])
                    wks.append(wk)

            if not timing_full:
                load_weights()

            # mm-only variant: constant stationary tile + scale, no quant path
            cxqt = cfs = None
            if variant == "mm":
                cxqt = consts.tile([P, KC, P], mybir.dt.bfloat16)
                nc.vector.memset(cxqt[:], 1.0)
                cfs = consts.tile([P, 1], mybir.dt.float32)
                nc.vector.memset(cfs[:], 1.0)

            # ---- main loop over token tiles
            def token_loop():
                if timing_full:
                    load_weights()
                for t in range(TT):
                    if variant == "mm":
                        mm_tile(t, cxqt, cfs)
                    else:
                        token_tile(t)

            def mm_tile(t, xqt, fs):
                ps = psum.tile([P, of], mybir.dt.float32)
                for k in range(KC):
                    for n in range(NOF):
                        nc.tensor.matmul(
                            ps[:, n * ns:(n + 1) * ns],
                            xqt[:, k, :],
                            wks[k][:, n * ns:(n + 1) * ns],
                            start=(k == 0), stop=(k == KC - 1))
                ot = outp.tile([P, of], mybir.dt.float32, name="ot_mm")
                for n in range(NOF):
                    nc.scalar.mul(ot[:, n * ns:(n + 1) * ns],
                                  ps[:, n * ns:(n + 1) * ns], fs[:])
                nc.sync.dma_start(out3[t], ot[:])

            def token_tile(t):
                # per-tile scalar vectors packed into one tile (SBUF slots
                # pad to 4KB/partition, so one tag instead of four)
                scv = scal.tile([P, 8], mybir.dt.float32, tag="scv",
                                name="scv")
                sc2 = scv[:, 0:2]
                s = scv[:, 2:3]
                inv = scv[:, 3:4]
                fs = scv[:, 4:5]
                # load x tile in halves, quantize
                xh = [None, None]
                for h in range(2):
                    if t == 0 and reps == 1 and pre_x:
                        xh[h] = pre_x[h]
                    else:
                        xh[h] = stage.tile([P, XH], mybir.dt.float32,
                                           tag="stage", name=f"xh{h}")
                        nc.sync.dma_start(xh[h][:],
                                          xf3[t][:, h * XH:(h + 1) * XH])
                    nc.vector.tensor_reduce(
                        sc2[:, h:h + 1], xh[h][:], axis=mybir.AxisListType.X,
                        op=mybir.AluOpType.max, apply_absolute_value=True)
                nc.vector.tensor_reduce(
                    s[:], sc2[:], axis=mybir.AxisListType.X,
                    op=mybir.AluOpType.max)
                nc.vector.tensor_scalar_max(s[:], s[:], 1e-5)
                nc.vector.reciprocal(inv[:], s[:])
                nc.vector.tensor_scalar_mul(inv[:], inv[:], 127.0)
                nc.vector.tensor_scalar(fs[:], s[:], wsb[:], 1.0 / 127.0,
                                        op0=mybir.AluOpType.mult,
                                        op1=mybir.AluOpType.mult)
                xq = xqp.tile([P, in_f], mybir.dt.bfloat16)
                for h in range(2):
                    xqs = xq[:, h * XH:(h + 1) * XH]
                    nc.vector.tensor_scalar(xh[h][:], xh[h][:], inv[:],
                                            MAGIC,
                                            op0=mybir.AluOpType.mult,
                                            op1=mybir.AluOpType.add)
                    nc.vector.tensor_scalar(xqs, xh[h][:], MAGIC, None,
                                            op0=mybir.AluOpType.subtract)

                # transpose xq [P, in_f] -> per-chunk [P, P] lhsT tiles
                xqt = xqtp.tile([P, KC, P], mybir.dt.bfloat16)
                nc.sync.dma_start_transpose(xqt[:], xq[:])

                # matmul: psum[tok, of] += xqt[k].T @ wk[k]
                ps = psum.tile([P, of], mybir.dt.float32)
                for k in range(KC):
                    for n in range(NOF):
                        nc.tensor.matmul(
                            ps[:, n * ns:(n + 1) * ns],
                            xqt[:, k, :],
                            wks[k][:, n * ns:(n + 1) * ns],
                            start=(k == 0), stop=(k == KC - 1))

                # evict with per-token scale, then store
                ot = outp.tile([P, of], mybir.dt.float32)
                for n in range(NOF):
                    nc.scalar.mul(ot[:, n * ns:(n + 1) * ns],
                                  ps[:, n * ns:(n + 1) * ns], fs[:])
                nc.sync.dma_start(out3[t], ot[:])

            if reps == 1:
                token_loop()
            else:
                with tc.For_i(0, reps, 1):
                    token_loop()
            if timing:
                nc.sync.dma_start(tiny[:], wsb[:])

    nc.compile()
    return nc


_CACHED = {}


def _get_program():
    if "nc" not in _CACHED:
        _CACHED["nc"] = build_program()
    return _CACHED["nc"]


def make_in_maps(x, weight_ternary, weight_scale):
    xf = np.ascontiguousarray(np.asarray(x).reshape(TOKENS, IN_F),
                              dtype=np.float32)
    wsb = np.full((P, 1), np.float32(np.asarray(weight_scale).reshape(-1)[0]),
                  dtype=np.float32)
    in_maps = []
    for c in range(N_CORES):
        shard = np.asarray(weight_ternary)[c * OF_SHARD:(c + 1) * OF_SHARD, :]
        # bf16 repack is lossless for ternary {-1,0,1}; transpose puts the
        # contraction dim on SBUF partitions with contiguous DMA rows
        wt_t = np.ascontiguousarray(shard.T).astype(ml_dtypes.bfloat16)
        in_maps.append({"x": xf, "wt": wt_t, "ws": wsb})
    return in_maps


def gather_out(results):
    full = np.empty((TOKENS, OUT_F), dtype=np.float32)
    for c in range(N_CORES):
        full[:, c * OF_SHARD:(c + 1) * OF_SHARD] = results[c]["out"]
    return full.reshape(B, S, OUT_F)


def kernel(x, weight_ternary, weight_scale):
    from concourse.bass_utils import run_bass_kernel_spmd

    nc = _get_program()
    in_maps = make_in_maps(x, weight_ternary, weight_scale)
    try:
        res = run_bass_kernel_spmd(nc, in_maps, list(range(N_CORES)))
    except Exception:
        # transient device/transport flakes: retry once
        import time as _time
        _time.sleep(5)
        res = run_bass_kernel_spmd(nc, in_maps, list(range(N_CORES)))
    return gather_out(res.results)


# revision 6
# speedup vs baseline: 1.2260x; 1.2260x over previous
"""BitLinear inference kernel for Trainium2 (8 NeuronCores, column-parallel).

Math (per reference):
  s[t]   = max(|x[t,:]|) clipped to >= 1e-5          (per-token scale)
  xq     = round(x / s * 127)  (round-half-even)      (int values in [-127,127])
  out    = (xq @ w_ternary.T) * (s * weight_scale / 127)

The integer matmul xq @ w.T is EXACT in bf16 x bf16 -> fp32 PSUM:
xq in [-127,127] and w in {-1,0,1} are exactly representable in bf16,
products are exact, and partial sums are < 2^24 so fp32 accumulation is
exact. Per-token dequant scale is applied to the fp32 PSUM output.

Sharding: column-parallel. weight rows (out_features) are sharded 8 ways;
x is replicated; outputs are concatenated on host along out_features.
The weight shard is shipped host-transposed AND pre-cast to bf16
([in_f, of_shard]) so it can be DMA'd straight into the resident SBUF
weight tiles — no on-device dequant/cast on the startup critical path.
Weight DMAs ride the ACT HWDGE ring (nc.scalar) so they don't
head-of-line-block the SP ring that carries x loads, xbar transposes
and output stores.

Per-core pipeline, per 128-token tile:
  DMA   x tile in (2 halves), per-tile DVE quant (abs-max reduce,
        reciprocal, mult+magic-add, magic-sub -> bf16),
  DMA   xbar transpose SBUF->SBUF (bf16) into [128, 32, 128] lhsT chunks,
  PE    32 LDW+128 matmuls (N=512) accumulating [128 tok, 2048 of] fp32
        across 2 double-buffered PSUM tiles (8 banks),
  ACT   per-token-scale eviction (activation Copy, scale=[128,1] AP),
  DMA   store.

Correctness vs the fp32 jax reference: norm relative error 2.3e-05
(from inv=127*(1/s) vs the reference's x/s*127 double-rounding; the
integer matmul itself is exact).
"""

import numpy as np
import ml_dtypes

import concourse.bass as bass
import concourse.mybir as mybir
import concourse.tile as tile
from concourse import bacc

P = 128
MAGIC = 12582912.0  # 1.5 * 2**23: (v + MAGIC) - MAGIC == round-half-even(v) for |v|<=2^21

# problem shapes (hardcoded per contract)
B, S, IN_F, OUT_F = 4, 2048, 4096, 16384
N_CORES = 8
TOKENS = B * S
OF_SHARD = OUT_F // N_CORES


def build_program(tokens=TOKENS, in_f=IN_F, of=OF_SHARD, n_devices=N_CORES,
                  debug=False, ns=512, reps=1, timing=False,
                  timing_full=False, deep=False, variant="full",
                  wring="act", wone=False, unroll=False):
    """Build the SPMD single-core program. Returns the compiled Bacc object.

    timing=True makes the big tensors internal (nothing shipped over the
    wire) and adds a tiny external in/out pair; reps>1 wraps the token loop
    in a hardware For_i so per-iteration time can be measured as a slope.
    timing_full=True additionally moves the weight load inside the rep
    loop, so the slope approximates full per-invocation device time
    (weight DMA included) rather than steady-state-tiles-only.
    """
    TT = tokens // P      # token tiles
    KC = in_f // P        # contraction chunks
    NOF = of // ns        # psum column slices
    XH = in_f // 2        # x staged in halves to save SBUF

    nc = bacc.Bacc("TRN2", target_bir_lowering=False, debug=debug,
                   num_devices=n_devices)

    big_kind = "Internal" if timing else "ExternalInput"
    xf = nc.dram_tensor("x", [tokens, in_f], mybir.dt.float32,
                        kind=big_kind).ap()
    wt = nc.dram_tensor("wt", [in_f, of], mybir.dt.bfloat16,
                        kind=big_kind).ap()
    ws = nc.dram_tensor("ws", [P, 1], mybir.dt.float32,
                        kind="ExternalInput").ap()
    out = nc.dram_tensor(
        "out", [tokens, of], mybir.dt.float32,
        kind="Internal" if timing else "ExternalOutput").ap()
    tiny = None
    if timing:
        tiny = nc.dram_tensor("tiny", [P, 1], mybir.dt.float32,
                              kind="ExternalOutput").ap()

    xf3 = xf.rearrange("(tt p) f -> tt p f", p=P)
    wt3 = wt.rearrange("(kc p) o -> kc p o", p=P)
    out3 = out.rearrange("(tt p) o -> tt p o", p=P)

    with tile.TileContext(nc) as tc:
        with (
            tc.tile_pool(name="consts", bufs=1) as consts,
            tc.tile_pool(name="wpool", bufs=1) as wpool,
            tc.tile_pool(name="stage", bufs=3) as stage,
            tc.tile_pool(name="xqp", bufs=2 if deep else 1) as xqp,
            tc.tile_pool(name="xqtp", bufs=3 if deep else 2) as xqtp,
            tc.tile_pool(name="outp", bufs=2) as outp,
            tc.tile_pool(name="scal", bufs=3) as scal,
            tc.tile_pool(name="psum", bufs=2, space="PSUM") as psum,
        ):
            wsb = consts.tile([P, 1], mybir.dt.float32)
            nc.sync.dma_start(wsb[:], ws[:])

            # tile 0's x loads first so they land at the SP queue head
            pre_x = []
            if reps == 1 and variant != "mm":
                for h in range(2):
                    xt = stage.tile([P, XH], mybir.dt.float32, tag="stage",
                                    name=f"prex{h}")
                    nc.sync.dma_start(xt[:], xf3[0][:, h * XH:(h + 1) * XH])
                    pre_x.append(xt)

            # ---- weights: bf16 [in_f, of] DMA'd straight into resident
            # SBUF chunks on the ACT HWDGE ring (keeps SP ring free for
            # the x/transpose/store pipeline).
            weng = nc.scalar if wring == "act" else nc.sync
            wks = []

            def load_weights():
                del wks[:]
                if wone:
                    # one big DMA: SBUF [P, KC, of] <- DRAM [(kc p), of]
                    wk_all = wpool.tile([P, KC, of], mybir.dt.bfloat16,
                                        tag="wk_all", name="wk_all")
                    weng.dma_start(wk_all[:],
                                   wt.rearrange("(kc p) o -> p kc o", p=P))
                    for k in range(KC):
                        wks.append(wk_all[:, k, :])
                    return
                for k in range(KC):
                    wk = wpool.tile([P, of], mybir.dt.bfloat16, tag=f"wk{k}",
                                    name=f"wk{k}")
                    weng.dma_start(wk[:], wt3[k])
                    wks.append(wk)

            if not timing_full:
                load_weights()

            # mm-only variant: constant stationary tile + scale, no quant path
            cxqt = cfs = None
            if variant == "mm":
                cxqt = consts.tile([P, KC, P], mybir.dt.bfloat16)
                nc.vector.memset(cxqt[:], 1.0)
                cfs = consts.tile([P, 1], mybir.dt.float32)
                nc.vector.memset(cfs[:], 1.0)

            # ---- main loop over token tiles
            def token_loop():
                if timing_full:
                    load_weights()
                for t in range(TT):
                    if variant == "mm":
                        mm_tile(t, cxqt, cfs)
                    else:
                        token_tile(t)

            def mm_tile(t, xqt, fs):
                ps = psum.tile([P, of], mybir.dt.float32)
                for k in range(KC):
                    for n in range(NOF):
                        nc.tensor.matmul(
                            ps[:, n * ns:(n + 1) * ns],
                            xqt[:, k, :],
                            wks[k][:, n * ns:(n + 1) * ns],
                            start=(k == 0), stop=(k == KC - 1))
                ot = outp.tile([P, of], mybir.dt.float32, name="ot_mm")
                for n in range(NOF):
                    nc.scalar.mul(ot[:, n * ns:(n + 1) * ns],
                                  ps[:, n * ns:(n + 1) * ns], fs[:])
                nc.sync.dma_start(out3[t], ot[:])

            def token_tile(t):
                # per-tile scalar vectors packed into one tile (SBUF slots
                # pad to 4KB/partition, so one tag instead of four)
                scv = scal.tile([P, 8], mybir.dt.float32, tag="scv",
                                name="scv")
                sc2 = scv[:, 0:2]
                s = scv[:, 2:3]
                inv = scv[:, 3:4]
                fs = scv[:, 4:5]
                # load x tile in halves, quantize
                xh = [None, None]
                for h in range(2):
                    if t == 0 and reps == 1 and pre_x:
                        xh[h] = pre_x[h]
                    else:
                        xh[h] = stage.tile([P, XH], mybir.dt.float32,
                                           tag="stage", name=f"xh{h}")
                        nc.sync.dma_start(xh[h][:],
                                          xf3[t][:, h * XH:(h + 1) * XH])
                    nc.vector.tensor_reduce(
                        sc2[:, h:h + 1], xh[h][:], axis=mybir.AxisListType.X,
                        op=mybir.AluOpType.max, apply_absolute_value=True)
                nc.vector.tensor_reduce(
                    s[:], sc2[:], axis=mybir.AxisListType.X,
                    op=mybir.AluOpType.max)
                nc.vector.tensor_scalar_max(s[:], s[:], 1e-5)
                nc.vector.reciprocal(inv[:], s[:])
                nc.vector.tensor_scalar_mul(inv[:], inv[:], 127.0)
                nc.vector.tensor_scalar(fs[:], s[:], wsb[:], 1.0 / 127.0,
                                        op0=mybir.AluOpType.mult,
                                        op1=mybir.AluOpType.mult)
                xq = xqp.tile([P, in_f], mybir.dt.bfloat16)
                for h in range(2):
                    xqs = xq[:, h * XH:(h + 1) * XH]
                    nc.vector.tensor_scalar(xh[h][:], xh[h][:], inv[:],
                                            MAGIC,
                                            op0=mybir.AluOpType.mult,
                                            op1=mybir.AluOpType.add)
                    nc.vector.tensor_scalar(xqs, xh[h][:], MAGIC, None,
                                            op0=mybir.AluOpType.subtract)

                # transpose xq [P, in_f] -> per-chunk [P, P] lhsT tiles
                xqt = xqtp.tile([P, KC, P], mybir.dt.bfloat16)
                nc.sync.dma_start_transpose(xqt[:], xq[:])

                # matmul: psum[tok, of] += xqt[k].T @ wk[k]
                ps = psum.tile([P, of], mybir.dt.float32)
                for k in range(KC):
                    for n in range(NOF):
                        nc.tensor.matmul(
                            ps[:, n * ns:(n + 1) * ns],
                            xqt[:, k, :],
                            wks[k][:, n * ns:(n + 1) * ns],
                            start=(k == 0), stop=(k == KC - 1))

                # evict with per-token scale, then store
                ot = outp.tile([P, of], mybir.dt.float32)
                for n in range(NOF):
                    nc.scalar.mul(ot[:, n * ns:(n + 1) * ns],
                                  ps[:, n * ns:(n + 1) * ns], fs[:])
                nc.sync.dma_start(out3[t], ot[:])

            if reps == 1:
                token_loop()
            elif unroll:
                for _ in range(reps):
                    token_loop()
            else:
                with tc.For_i(0, reps, 1):
                    token_loop()
            if timing:
                nc.sync.dma_start(tiny[:], wsb[:])

    nc.compile()
    return nc


_CACHED = {}


def _get_program():
    if "nc" not in _CACHED:
        _CACHED["nc"] = build_program()
    return _CACHED["nc"]


def make_in_maps(x, weight_ternary, weight_scale):
    xf = np.ascontiguousarray(np.asarray(x).reshape(TOKENS, IN_F),
                              dtype=np.float32)
    wsb = np.full((P, 1), np.float32(np.asarray(weight_scale).reshape(-1)[0]),
                  dtype=np.float32)
    in_maps = []
    for c in range(N_CORES):
        shard = np.asarray(weight_ternary)[c * OF_SHARD:(c + 1) * OF_SHARD, :]
        # bf16 repack is lossless for ternary {-1,0,1}; transpose puts the
        # contraction dim on SBUF partitions with contiguous DMA rows
        wt_t = np.ascontiguousarray(shard.T).astype(ml_dtypes.bfloat16)
        in_maps.append({"x": xf, "wt": wt_t, "ws": wsb})
    return in_maps


def gather_out(results):
    full = np.empty((TOKENS, OUT_F), dtype=np.float32)
    for c in range(N_CORES):
        full[:, c * OF_SHARD:(c + 1) * OF_SHARD] = results[c]["out"]
    return full.reshape(B, S, OUT_F)


def kernel(x, weight_ternary, weight_scale):
    from concourse.bass_utils import run_bass_kernel_spmd

    nc = _get_program()
    in_maps = make_in_maps(x, weight_ternary, weight_scale)
    try:
        res = run_bass_kernel_spmd(nc, in_maps, list(range(N_CORES)))
    except Exception:
        # transient device/transport flakes: retry once
        import time as _time
        _time.sleep(5)
        res = run_bass_kernel_spmd(nc, in_maps, list(range(N_CORES)))
    return gather_out(res.results)


# revision 8
# speedup vs baseline: 1.2870x; 1.0498x over previous
"""BitLinear inference kernel for Trainium2 (8 NeuronCores, column-parallel).

Math (per reference):
  s[t]   = max(|x[t,:]|) clipped to >= 1e-5          (per-token scale)
  xq     = round(x / s * 127)  (round-half-even)      (int values in [-127,127])
  out    = (xq @ w_ternary.T) * (s * weight_scale / 127)

The integer matmul xq @ w.T is EXACT in bf16 x bf16 -> fp32 PSUM:
xq in [-127,127] and w in {-1,0,1} are exactly representable in bf16,
products are exact, and partial sums are < 2^24 so fp32 accumulation is
exact. Per-token dequant scale is applied to the fp32 PSUM output.

Sharding: column-parallel. weight rows (out_features) are sharded 8 ways;
x is replicated; outputs are concatenated on host along out_features.
The weight shard is shipped host-transposed AND pre-cast to bf16
([in_f, of_shard]) so it can be DMA'd straight into the resident SBUF
weight tiles — no on-device dequant/cast on the startup critical path.
Weight DMAs ride the ACT HWDGE ring (nc.scalar) so they don't
head-of-line-block the SP ring that carries x loads, xbar transposes
and output stores.

Per-core pipeline, per 128-token tile:
  DMA   x tile in (2 halves), per-tile DVE quant (abs-max reduce,
        reciprocal, mult+magic-add, magic-sub -> bf16),
  DMA   xbar transpose SBUF->SBUF (bf16) into [128, 32, 128] lhsT chunks,
  PE    32 LDW+128 matmuls (N=512) accumulating [128 tok, 2048 of] fp32
        across 2 double-buffered PSUM tiles (8 banks),
  ACT   per-token-scale eviction (activation Copy, scale=[128,1] AP),
  DMA   store.

Correctness vs the fp32 jax reference: norm relative error 2.3e-05
(from inv=127*(1/s) vs the reference's x/s*127 double-rounding; the
integer matmul itself is exact).
"""

import numpy as np
import ml_dtypes

import concourse.bass as bass
import concourse.mybir as mybir
import concourse.tile as tile
from concourse import bacc

P = 128
MAGIC = 12582912.0  # 1.5 * 2**23: (v + MAGIC) - MAGIC == round-half-even(v) for |v|<=2^21

# problem shapes (hardcoded per contract)
B, S, IN_F, OUT_F = 4, 2048, 4096, 16384
N_CORES = 8
TOKENS = B * S
OF_SHARD = OUT_F // N_CORES


def build_program(tokens=TOKENS, in_f=IN_F, of=OF_SHARD, n_devices=N_CORES,
                  debug=False, ns=512, reps=1, timing=False,
                  timing_full=False, deep=False, variant="full",
                  wring="act", wgroups=4, unroll=False):
    """Build the SPMD single-core program. Returns the compiled Bacc object.

    timing=True makes the big tensors internal (nothing shipped over the
    wire) and adds a tiny external in/out pair; reps>1 wraps the token loop
    in a hardware For_i so per-iteration time can be measured as a slope.
    timing_full=True additionally moves the weight load inside the rep
    loop, so the slope approximates full per-invocation device time
    (weight DMA included) rather than steady-state-tiles-only.
    """
    TT = tokens // P      # token tiles
    KC = in_f // P        # contraction chunks
    NOF = of // ns        # psum column slices
    XH = in_f // 2        # x staged in halves to save SBUF

    nc = bacc.Bacc("TRN2", target_bir_lowering=False, debug=debug,
                   num_devices=n_devices)

    big_kind = "Internal" if timing else "ExternalInput"
    xf = nc.dram_tensor("x", [tokens, in_f], mybir.dt.float32,
                        kind=big_kind).ap()
    wt = nc.dram_tensor("wt", [in_f, of], mybir.dt.bfloat16,
                        kind=big_kind).ap()
    ws = nc.dram_tensor("ws", [P, 1], mybir.dt.float32,
                        kind="ExternalInput").ap()
    out = nc.dram_tensor(
        "out", [tokens, of], mybir.dt.float32,
        kind="Internal" if timing else "ExternalOutput").ap()
    tiny = None
    if timing:
        tiny = nc.dram_tensor("tiny", [P, 1], mybir.dt.float32,
                              kind="ExternalOutput").ap()

    xf3 = xf.rearrange("(tt p) f -> tt p f", p=P)
    wt3 = wt.rearrange("(kc p) o -> kc p o", p=P)
    out3 = out.rearrange("(tt p) o -> tt p o", p=P)

    with tile.TileContext(nc) as tc:
        with (
            tc.tile_pool(name="consts", bufs=1) as consts,
            tc.tile_pool(name="wpool", bufs=1) as wpool,
            tc.tile_pool(name="stage", bufs=3) as stage,
            tc.tile_pool(name="xqp", bufs=2 if deep else 1) as xqp,
            tc.tile_pool(name="xqtp", bufs=3 if deep else 2) as xqtp,
            tc.tile_pool(name="outp", bufs=2) as outp,
            tc.tile_pool(name="scal", bufs=3) as scal,
            tc.tile_pool(name="psum", bufs=2, space="PSUM") as psum,
        ):
            wsb = consts.tile([P, 1], mybir.dt.float32)
            nc.sync.dma_start(wsb[:], ws[:])

            # tile 0's x loads first so they land at the SP queue head
            pre_x = []
            if reps == 1 and variant != "mm":
                for h in range(2):
                    xt = stage.tile([P, XH], mybir.dt.float32, tag="stage",
                                    name=f"prex{h}")
                    nc.sync.dma_start(xt[:], xf3[0][:, h * XH:(h + 1) * XH])
                    pre_x.append(xt)

            # ---- weights: bf16 [in_f, of] DMA'd straight into resident
            # SBUF chunks on the ACT HWDGE ring (keeps SP ring free for
            # the x/transpose/store pipeline).
            weng = nc.scalar if wring == "act" else nc.sync
            wks = []

            def load_weights():
                # a few large DMAs: chunked small DMAs pay a big per-DMA
                # penalty on HW (measured ~+500us for 32 chunks vs 1 big),
                # while one giant DMA delays the first matmul until all
                # 16MiB land. Groups balance the two.
                del wks[:]
                wk_all = wpool.tile([P, KC, of], mybir.dt.bfloat16,
                                    tag="wk_all", name="wk_all")
                src = wt.rearrange("(kc p) o -> p kc o", p=P)
                gsz = KC // wgroups
                for g in range(wgroups):
                    weng.dma_start(wk_all[:, g * gsz:(g + 1) * gsz, :],
                                   src[:, g * gsz:(g + 1) * gsz, :])
                for k in range(KC):
                    wks.append(wk_all[:, k, :])

            if not timing_full:
                load_weights()

            # mm-only variant: constant stationary tile + scale, no quant path
            cxqt = cfs = None
            if variant == "mm":
                cxqt = consts.tile([P, KC, P], mybir.dt.bfloat16)
                nc.vector.memset(cxqt[:], 1.0)
                cfs = consts.tile([P, 1], mybir.dt.float32)
                nc.vector.memset(cfs[:], 1.0)

            # ---- main loop over token tiles
            def token_loop():
                if timing_full:
                    load_weights()
                for t in range(TT):
                    if variant == "mm":
                        mm_tile(t, cxqt, cfs)
                    else:
                        token_tile(t)

            def mm_tile(t, xqt, fs):
                ps = psum.tile([P, of], mybir.dt.float32)
                for k in range(KC):
                    for n in range(NOF):
                        nc.tensor.matmul(
                            ps[:, n * ns:(n + 1) * ns],
                            xqt[:, k, :],
                            wks[k][:, n * ns:(n + 1) * ns],
                            start=(k == 0), stop=(k == KC - 1))
                ot = outp.tile([P, of], mybir.dt.float32, name="ot_mm")
                for n in range(NOF):
                    nc.scalar.mul(ot[:, n * ns:(n + 1) * ns],
                                  ps[:, n * ns:(n + 1) * ns], fs[:])
                nc.sync.dma_start(out3[t], ot[:])

            def token_tile(t):
                # per-tile scalar vectors packed into one tile (SBUF slots
                # pad to 4KB/partition, so one tag instead of four)
                scv = scal.tile([P, 8], mybir.dt.float32, tag="scv",
                                name="scv")
                sc2 = scv[:, 0:2]
                s = scv[:, 2:3]
                inv = scv[:, 3:4]
                fs = scv[:, 4:5]
                # load x tile in halves, quantize
                xh = [None, None]
                for h in range(2):
                    if t == 0 and reps == 1 and pre_x:
                        xh[h] = pre_x[h]
                    else:
                        xh[h] = stage.tile([P, XH], mybir.dt.float32,
                                           tag="stage", name=f"xh{h}")
                        nc.sync.dma_start(xh[h][:],
                                          xf3[t][:, h * XH:(h + 1) * XH])
                    nc.vector.tensor_reduce(
                        sc2[:, h:h + 1], xh[h][:], axis=mybir.AxisListType.X,
                        op=mybir.AluOpType.max, apply_absolute_value=True)
                nc.vector.tensor_reduce(
                    s[:], sc2[:], axis=mybir.AxisListType.X,
                    op=mybir.AluOpType.max)
                nc.vector.tensor_scalar_max(s[:], s[:], 1e-5)
                nc.vector.reciprocal(inv[:], s[:])
                nc.vector.tensor_scalar_mul(inv[:], inv[:], 127.0)
                nc.vector.tensor_scalar(fs[:], s[:], wsb[:], 1.0 / 127.0,
                                        op0=mybir.AluOpType.mult,
                                        op1=mybir.AluOpType.mult)
                xq = xqp.tile([P, in_f], mybir.dt.bfloat16)
                for h in range(2):
                    xqs = xq[:, h * XH:(h + 1) * XH]
                    nc.vector.tensor_scalar(xh[h][:], xh[h][:], inv[:],
                                            MAGIC,
                                            op0=mybir.AluOpType.mult,
                                            op1=mybir.AluOpType.add)
                    nc.vector.tensor_scalar(xqs, xh[h][:], MAGIC, None,
                                            op0=mybir.AluOpType.subtract)

                # transpose xq [P, in_f] -> per-chunk [P, P] lhsT tiles
                xqt = xqtp.tile([P, KC, P], mybir.dt.bfloat16)
                nc.sync.dma_start_transpose(xqt[:], xq[:])

                # matmul: psum[tok, of] += xqt[k].T @ wk[k]
                ps = psum.tile([P, of], mybir.dt.float32)
                for k in range(KC):
                    for n in range(NOF):
                        nc.tensor.matmul(
                            ps[:, n * ns:(n + 1) * ns],
                            xqt[:, k, :],
                            wks[k][:, n * ns:(n + 1) * ns],
                            start=(k == 0), stop=(k == KC - 1))

                # evict with per-token scale, then store
                ot = outp.tile([P, of], mybir.dt.float32)
                for n in range(NOF):
                    nc.scalar.mul(ot[:, n * ns:(n + 1) * ns],
                                  ps[:, n * ns:(n + 1) * ns], fs[:])
                nc.sync.dma_start(out3[t], ot[:])

            if reps == 1:
                token_loop()
            elif unroll:
                for _ in range(reps):
                    token_loop()
            else:
                with tc.For_i(0, reps, 1):
                    token_loop()
            if timing:
                nc.sync.dma_start(tiny[:], wsb[:])

    nc.compile()
    return nc


_CACHED = {}


def _get_program():
    if "nc" not in _CACHED:
        _CACHED["nc"] = build_program()
    return _CACHED["nc"]


def make_in_maps(x, weight_ternary, weight_scale):
    xf = np.ascontiguousarray(np.asarray(x).reshape(TOKENS, IN_F),
                              dtype=np.float32)
    wsb = np.full((P, 1), np.float32(np.asarray(weight_scale).reshape(-1)[0]),
                  dtype=np.float32)
    in_maps = []
    for c in range(N_CORES):
        shard = np.asarray(weight_ternary)[c * OF_SHARD:(c + 1) * OF_SHARD, :]
        # bf16 repack is lossless for ternary {-1,0,1}; transpose puts the
        # contraction dim on SBUF partitions with contiguous DMA rows
        wt_t = np.ascontiguousarray(shard.T).astype(ml_dtypes.bfloat16)
        in_maps.append({"x": xf, "wt": wt_t, "ws": wsb})
    return in_maps


def gather_out(results):
    full = np.empty((TOKENS, OUT_F), dtype=np.float32)
    for c in range(N_CORES):
        full[:, c * OF_SHARD:(c + 1) * OF_SHARD] = results[c]["out"]
    return full.reshape(B, S, OUT_F)


def kernel(x, weight_ternary, weight_scale):
    from concourse.bass_utils import run_bass_kernel_spmd

    nc = _get_program()
    in_maps = make_in_maps(x, weight_ternary, weight_scale)
    try:
        res = run_bass_kernel_spmd(nc, in_maps, list(range(N_CORES)))
    except Exception:
        # transient device/transport flakes: retry once
        import time as _time
        _time.sleep(5)
        res = run_bass_kernel_spmd(nc, in_maps, list(range(N_CORES)))
    return gather_out(res.results)


# revision 9
# speedup vs baseline: 1.3255x; 1.0299x over previous
"""BitLinear inference kernel for Trainium2 (8 NeuronCores, column-parallel).

Math (per reference):
  s[t]   = max(|x[t,:]|) clipped to >= 1e-5          (per-token scale)
  xq     = round(x / s * 127)  (round-half-even)      (int values in [-127,127])
  out    = (xq @ w_ternary.T) * (s * weight_scale / 127)

The integer matmul xq @ w.T is EXACT in bf16 x bf16 -> fp32 PSUM:
xq in [-127,127] and w in {-1,0,1} are exactly representable in bf16,
products are exact, and partial sums are < 2^24 so fp32 accumulation is
exact. Per-token dequant scale is applied to the fp32 PSUM output.

Sharding: column-parallel. weight rows (out_features) are sharded 8 ways;
x is replicated; outputs are concatenated on host along out_features.
The weight shard is shipped host-transposed AND pre-cast to bf16
([in_f, of_shard]) so it can be DMA'd straight into the resident SBUF
weight tiles — no on-device dequant/cast on the startup critical path.
Weight DMAs ride the ACT HWDGE ring (nc.scalar) so they don't
head-of-line-block the SP ring that carries x loads, xbar transposes
and output stores.

Per-core pipeline, per 128-token tile:
  DMA   x tile in (2 halves), per-tile DVE quant (abs-max reduce,
        reciprocal, mult+magic-add, magic-sub -> bf16),
  DMA   xbar transpose SBUF->SBUF (bf16) into [128, 32, 128] lhsT chunks,
  PE    32 LDW+128 matmuls (N=512) accumulating [128 tok, 2048 of] fp32
        across 2 double-buffered PSUM tiles (8 banks),
  ACT   per-token-scale eviction (activation Copy, scale=[128,1] AP),
  DMA   store.

Correctness vs the fp32 jax reference: norm relative error 2.3e-05
(from inv=127*(1/s) vs the reference's x/s*127 double-rounding; the
integer matmul itself is exact).
"""

import numpy as np
import ml_dtypes

import concourse.bass as bass
import concourse.mybir as mybir
import concourse.tile as tile
from concourse import bacc

P = 128
MAGIC = 12582912.0  # 1.5 * 2**23: (v + MAGIC) - MAGIC == round-half-even(v) for |v|<=2^21

# problem shapes (hardcoded per contract)
B, S, IN_F, OUT_F = 4, 2048, 4096, 16384
N_CORES = 8
TOKENS = B * S
OF_SHARD = OUT_F // N_CORES


def build_program(tokens=TOKENS, in_f=IN_F, of=OF_SHARD, n_devices=N_CORES,
                  debug=False, ns=512, reps=1, timing=False,
                  timing_full=False, deep=False, variant="full",
                  wring="act", wgroups=4, unroll=False):
    """Build the SPMD single-core program. Returns the compiled Bacc object.

    timing=True makes the big tensors internal (nothing shipped over the
    wire) and adds a tiny external in/out pair; reps>1 wraps the token loop
    in a hardware For_i so per-iteration time can be measured as a slope.
    timing_full=True additionally moves the weight load inside the rep
    loop, so the slope approximates full per-invocation device time
    (weight DMA included) rather than steady-state-tiles-only.
    """
    TT = tokens // P      # token tiles
    KC = in_f // P        # contraction chunks
    NOF = of // ns        # psum column slices
    XH = in_f // 2        # x staged in halves to save SBUF

    nc = bacc.Bacc("TRN2", target_bir_lowering=False, debug=debug,
                   num_devices=n_devices)

    big_kind = "Internal" if timing else "ExternalInput"
    xf = nc.dram_tensor("x", [tokens, in_f], mybir.dt.float32,
                        kind=big_kind).ap()
    wt = nc.dram_tensor("wt", [in_f, of], mybir.dt.bfloat16,
                        kind=big_kind).ap()
    ws = nc.dram_tensor("ws", [P, 1], mybir.dt.float32,
                        kind="ExternalInput").ap()
    out = nc.dram_tensor(
        "out", [tokens, of], mybir.dt.float32,
        kind="Internal" if timing else "ExternalOutput").ap()
    tiny = None
    if timing:
        tiny = nc.dram_tensor("tiny", [P, 1], mybir.dt.float32,
                              kind="ExternalOutput").ap()

    xf3 = xf.rearrange("(tt p) f -> tt p f", p=P)
    wt3 = wt.rearrange("(kc p) o -> kc p o", p=P)
    out3 = out.rearrange("(tt p) o -> tt p o", p=P)

    with tile.TileContext(nc) as tc:
        with (
            tc.tile_pool(name="consts", bufs=1) as consts,
            tc.tile_pool(name="wpool", bufs=1) as wpool,
            tc.tile_pool(name="stage", bufs=2 if deep else 3) as stage,
            tc.tile_pool(name="xqp", bufs=2 if deep else 1) as xqp,
            tc.tile_pool(name="xqtp", bufs=3 if deep else 2) as xqtp,
            tc.tile_pool(name="outp", bufs=2) as outp,
            tc.tile_pool(name="scal", bufs=3) as scal,
            tc.tile_pool(name="psum", bufs=2, space="PSUM") as psum,
        ):
            wsb = consts.tile([P, 1], mybir.dt.float32)
            nc.sync.dma_start(wsb[:], ws[:])

            # tile 0's x loads first so they land at the SP queue head
            pre_x = []
            if reps == 1 and variant != "mm":
                for h in range(2):
                    xt = stage.tile([P, XH], mybir.dt.float32, tag="stage",
                                    name=f"prex{h}")
                    nc.sync.dma_start(xt[:], xf3[0][:, h * XH:(h + 1) * XH])
                    pre_x.append(xt)

            # ---- weights: bf16 [in_f, of] DMA'd straight into resident
            # SBUF chunks on the ACT HWDGE ring (keeps SP ring free for
            # the x/transpose/store pipeline).
            weng = nc.scalar if wring == "act" else nc.sync
            wks = []

            def load_weights():
                # a few large DMAs: chunked small DMAs pay a big per-DMA
                # penalty on HW (measured ~+500us for 32 chunks vs 1 big),
                # while one giant DMA delays the first matmul until all
                # 16MiB land. Groups balance the two.
                del wks[:]
                wk_all = wpool.tile([P, KC, of], mybir.dt.bfloat16,
                                    tag="wk_all", name="wk_all")
                src = wt.rearrange("(kc p) o -> p kc o", p=P)
                gsz = KC // wgroups
                for g in range(wgroups):
                    weng.dma_start(wk_all[:, g * gsz:(g + 1) * gsz, :],
                                   src[:, g * gsz:(g + 1) * gsz, :])
                for k in range(KC):
                    wks.append(wk_all[:, k, :])

            if not timing_full:
                load_weights()

            # mm-only variant: constant stationary tile + scale, no quant path
            cxqt = cfs = None
            if variant == "mm":
                cxqt = consts.tile([P, KC, P], mybir.dt.bfloat16)
                nc.vector.memset(cxqt[:], 1.0)
                cfs = consts.tile([P, 1], mybir.dt.float32)
                nc.vector.memset(cfs[:], 1.0)

            # ---- main loop over token tiles
            def token_loop():
                if timing_full:
                    load_weights()
                for t in range(TT):
                    if variant == "mm":
                        mm_tile(t, cxqt, cfs)
                    else:
                        token_tile(t)

            def mm_tile(t, xqt, fs):
                ps = psum.tile([P, of], mybir.dt.float32)
                for k in range(KC):
                    for n in range(NOF):
                        nc.tensor.matmul(
                            ps[:, n * ns:(n + 1) * ns],
                            xqt[:, k, :],
                            wks[k][:, n * ns:(n + 1) * ns],
                            start=(k == 0), stop=(k == KC - 1))
                ot = outp.tile([P, of], mybir.dt.float32, name="ot_mm")
                for n in range(NOF):
                    nc.scalar.mul(ot[:, n * ns:(n + 1) * ns],
                                  ps[:, n * ns:(n + 1) * ns], fs[:])
                nc.sync.dma_start(out3[t], ot[:])

            def token_tile(t):
                # per-tile scalar vectors packed into one tile (SBUF slots
                # pad to 4KB/partition, so one tag instead of four)
                scv = scal.tile([P, 8], mybir.dt.float32, tag="scv",
                                name="scv")
                sc2 = scv[:, 0:2]
                s = scv[:, 2:3]
                inv = scv[:, 3:4]
                fs = scv[:, 4:5]
                # load x tile in halves, quantize
                xh = [None, None]
                for h in range(2):
                    if t == 0 and reps == 1 and pre_x:
                        xh[h] = pre_x[h]
                    else:
                        xh[h] = stage.tile([P, XH], mybir.dt.float32,
                                           tag="stage", name=f"xh{h}")
                        nc.sync.dma_start(xh[h][:],
                                          xf3[t][:, h * XH:(h + 1) * XH])
                    nc.vector.tensor_reduce(
                        sc2[:, h:h + 1], xh[h][:], axis=mybir.AxisListType.X,
                        op=mybir.AluOpType.max, apply_absolute_value=True)
                nc.vector.tensor_reduce(
                    s[:], sc2[:], axis=mybir.AxisListType.X,
                    op=mybir.AluOpType.max)
                nc.vector.tensor_scalar_max(s[:], s[:], 1e-5)
                nc.vector.reciprocal(inv[:], s[:])
                nc.vector.tensor_scalar_mul(inv[:], inv[:], 127.0)
                nc.vector.tensor_scalar(fs[:], s[:], wsb[:], 1.0 / 127.0,
                                        op0=mybir.AluOpType.mult,
                                        op1=mybir.AluOpType.mult)
                xq = xqp.tile([P, in_f], mybir.dt.bfloat16)
                for h in range(2):
                    xqs = xq[:, h * XH:(h + 1) * XH]
                    nc.vector.tensor_scalar(xh[h][:], xh[h][:], inv[:],
                                            MAGIC,
                                            op0=mybir.AluOpType.mult,
                                            op1=mybir.AluOpType.add)
                    nc.vector.tensor_scalar(xqs, xh[h][:], MAGIC, None,
                                            op0=mybir.AluOpType.subtract)

                # transpose xq [P, in_f] -> per-chunk [P, P] lhsT tiles
                xqt = xqtp.tile([P, KC, P], mybir.dt.bfloat16)
                nc.sync.dma_start_transpose(xqt[:], xq[:])

                # matmul: psum[tok, of] += xqt[k].T @ wk[k]
                ps = psum.tile([P, of], mybir.dt.float32)
                for k in range(KC):
                    for n in range(NOF):
                        nc.tensor.matmul(
                            ps[:, n * ns:(n + 1) * ns],
                            xqt[:, k, :],
                            wks[k][:, n * ns:(n + 1) * ns],
                            start=(k == 0), stop=(k == KC - 1))

                # evict with per-token scale, then store
                ot = outp.tile([P, of], mybir.dt.float32)
                for n in range(NOF):
                    nc.scalar.mul(ot[:, n * ns:(n + 1) * ns],
                                  ps[:, n * ns:(n + 1) * ns], fs[:])
                nc.sync.dma_start(out3[t], ot[:])

            if reps == 1:
                token_loop()
            elif unroll:
                for _ in range(reps):
                    token_loop()
            else:
                with tc.For_i(0, reps, 1):
                    token_loop()
            if timing:
                nc.sync.dma_start(tiny[:], wsb[:])

    nc.compile()
    return nc


_CACHED = {}


def _get_program():
    if "nc" not in _CACHED:
        _CACHED["nc"] = build_program()
    return _CACHED["nc"]


def make_in_maps(x, weight_ternary, weight_scale):
    xf = np.ascontiguousarray(np.asarray(x).reshape(TOKENS, IN_F),
                              dtype=np.float32)
    wsb = np.full((P, 1), np.float32(np.asarray(weight_scale).reshape(-1)[0]),
                  dtype=np.float32)
    in_maps = []
    for c in range(N_CORES):
        shard = np.asarray(weight_ternary)[c * OF_SHARD:(c + 1) * OF_SHARD, :]
        # bf16 repack is lossless for ternary {-1,0,1}; transpose puts the
        # contraction dim on SBUF partitions with contiguous DMA rows
        wt_t = np.ascontiguousarray(shard.T).astype(ml_dtypes.bfloat16)
        in_maps.append({"x": xf, "wt": wt_t, "ws": wsb})
    return in_maps


def gather_out(results):
    full = np.empty((TOKENS, OUT_F), dtype=np.float32)
    for c in range(N_CORES):
        full[:, c * OF_SHARD:(c + 1) * OF_SHARD] = results[c]["out"]
    return full.reshape(B, S, OUT_F)


def kernel(x, weight_ternary, weight_scale):
    from concourse.bass_utils import run_bass_kernel_spmd

    nc = _get_program()
    in_maps = make_in_maps(x, weight_ternary, weight_scale)
    try:
        res = run_bass_kernel_spmd(nc, in_maps, list(range(N_CORES)))
    except Exception:
        # transient device/transport flakes: retry once
        import time as _time
        _time.sleep(5)
        res = run_bass_kernel_spmd(nc, in_maps, list(range(N_CORES)))
    return gather_out(res.results)
